# revision 28
# baseline (speedup 1.0000x reference)
# Multi-head attention kernel for Trainium2, sharded over 8 NeuronCores.
#
# Sharding: core = (batch b, query-chunk qc). Each core handles QB=512 queries
# of one batch, all 12 heads, recomputing the K/V projections for its batch.
# (Cross-core collectives measured far too slow on this fabric: an AllGather
# of the projected K/V costs ~30us + ~50us launch overhead, much more than
# the redundant projection compute it would save.)
#
# Numerics (rel err budget 2e-2; measures ~9.5e-3 on HW):
#   - q/k path in fp8e4 with DoubleRow matmuls: inputs xq/xk and weights
#     wq/wk are fp8; weights scaled x64 to dodge the e4m3 subnormal range,
#     q stored x8; the 8x score scale is removed for free via the exp
#     activation's scale=1/8.
#   - v path, PV, and output projection stay bf16: fp8 noise on v/ex/o hits
#     the output undamped (sims at 1.5e-2+), so fp8 there is not safe.
#   - d-split layout: the host permutes W columns so projection PSUM rows
#     land directly as [32-partition blocks x 2 planes] (head = 3*g + m,
#     bases 0/32/64 - base 96 is not encodable for matmul operands); score
#     matmuls then contract K=64 as DoubleRow [32p x 2 planes], and the two
#     heads of a pair run CONCURRENTLY on disjoint PE row groups.
#
# Schedule (exp is ~107us of Scalar work; PE stream is ~150us and is the
# binding engine, so the goal is to keep PE saturated and never let its
# in-order stream wait on the Scalar engine):
#   - all inputs are host-packed into SBUF layouts so each is ONE DMA of
#     long per-partition runs (sub-KB-run DMAs measured ~47GB/s); ~24 dummy
#     warm-up matmuls bridge the input-DMA latency so the PE clock ramp
#     (0.65->2.4GHz, reset by any idle moment) is complete when real work
#     starts.
#   - kt-blocked attention (blocks of 4/6/6 key tiles x 6 head pairs):
#     o (+ ones-column denominator row) accumulates in PSUM per block and is
#     flushed into an SBUF f32 accumulator on DVE.  PSUM: 3 score slots
#     [128,2,512] + 2 o slots [65,512] = 8 banks.
#   - within a head pair, PV is software-pipelined one key tile behind
#     scores/exp; remaining projection work (v tiles, late q/k tiles) sits
#     in a FIFO queue pumped between score groups, with idempotent need_*
#     helpers as deadline enforcement.  FIFO order guarantees a block's v
#     tiles are emitted before that block's PVs.
#   - softmax normalization (denominator gather + one batched reciprocal
#     per 6 heads + PE selector-broadcast + DVE multiply) and the first
#     half of the output projection are emitted inside the last block's
#     Scalar-bound stretch, so the tail after the final exp is only the
#     second half of the output projection.
#
import numpy as np
from contextlib import ExitStack

import concourse.bass as bass
import concourse.mybir as mybir
import concourse.tile as tile
from concourse import bacc
from concourse.bass_utils import run_bass_kernel_spmd

F32 = mybir.dt.float32
BF16 = mybir.dt.bfloat16
F8 = mybir.dt.float8e4
DR = mybir.MatmulPerfMode.DoubleRow
P = 128
E = 768
S = 2048
B = 2
H = 12
D = 64
QB = 512          # queries per core
NCORES = 8
EC = E // P       # 6 e-chunks
KT = S // P       # 16 key tiles
MT = E // P       # 6 e-chunks of proj output
NG = 4            # d-split head groups (head = 3*g + m, m in 0..2)
PT = 8            # projection tiles of 96 rows (bases 96+ are not encodable)
PR = 96           # rows per projection tile
NC4 = S // 512    # 4 n-slices of k
KB = 4            # key tiles per attention block
NBLK = KT // KB   # 4 blocks
WS = 64.0         # fp8 weight scale (avoids e4m3 subnormals)
QS = 8.0          # qT8 storage scale; removed via exp scale


def build_nc():
    nc = bacc.Bacc("TRN2", debug=False)

    # all inputs pre-packed host-side into SBUF layouts so every DMA moves
    # long contiguous runs per partition (sub-KB runs measure ~47GB/s)
    xq = nc.dram_tensor("xq", (P, EC, QB), F8, kind="ExternalInput")
    xk = nc.dram_tensor("xk", (P, EC, S), F8, kind="ExternalInput")
    xv = nc.dram_tensor("xv", (P, KT, EC, P), BF16, kind="ExternalInput")
    wq = nc.dram_tensor("wq", (P, EC, E), F8, kind="ExternalInput")   # d-split cols, x(1/sqrt(D))x64
    wk = nc.dram_tensor("wk", (P, EC, E), F8, kind="ExternalInput")   # d-split cols, x64
    wv = nc.dram_tensor("wv", (P, EC, E), BF16, kind="ExternalInput")
    wo = nc.dram_tensor("wo", (P, EC, E), BF16, kind="ExternalInput")
    bq = nc.dram_tensor("bq", (P, PT), F32, kind="ExternalInput")   # d-split rows, x8
    bk = nc.dram_tensor("bk", (P, PT), F32, kind="ExternalInput")   # d-split rows
    bo = nc.dram_tensor("bo", (P, E), F32, kind="ExternalInput")    # bv@Wo + bo, broadcast
    seld = nc.dram_tensor("seld", (66, H * D), BF16, kind="ExternalInput")  # head-broadcast selector
    out = nc.dram_tensor("out", (QB, E), F32, kind="ExternalOutput")

    with tile.TileContext(nc) as tc:
        with ExitStack() as ctx:
            _emit(ctx, tc, nc, xq, xk, xv, wq, wk, wv, wo, bq, bk, bo, seld, out)
    nc.compile()
    return nc


def _emit(ctx, tc, nc, xq, xk, xv, wq, wk, wv, wo, bq, bk, bo, seld, out):
    mult = mybir.AluOpType.mult
    add = mybir.AluOpType.add

    persist = ctx.enter_context(tc.tile_pool(name="persist", bufs=1))
    wpool = ctx.enter_context(tc.tile_pool(name="wpool", bufs=1))
    xpool = ctx.enter_context(tc.tile_pool(name="xpool", bufs=1))
    xvpool = ctx.enter_context(tc.tile_pool(name="xvpool", bufs=3))
    expool = ctx.enter_context(tc.tile_pool(name="expool", bufs=16))
    outpool = ctx.enter_context(tc.tile_pool(name="outpool", bufs=2))
    psS = ctx.enter_context(tc.tile_pool(name="psS", bufs=3, space="PSUM"))  # [128,2,512] scores/proj
    psO = ctx.enter_context(tc.tile_pool(name="psO", bufs=2, space="PSUM"))  # [65,512] o accum / bc

    # persistent SBUF
    qT8 = persist.tile([P, 2, NG, QB], F8)         # [32m+dm, plane j, group g, q] = 8*q
    kT8 = persist.tile([P, 2, NG, S], F8)          # [32m+dm, j, g, key] = k
    v_sb = persist.tile([P, KT, H, D + 1], BF16)   # v + ones column per head
    o_acc = persist.tile([D + 1, H, QB], F32)      # flushed o (+denominator in row D)
    o_all = persist.tile([P, H // 2, QB], BF16)    # normalized o, pairs in partition halves
    dens = persist.tile([66, QB], F32)             # denominators: rows 0-5 / 32-35 / 64-65
    drec2 = persist.tile([66, QB], BF16)           # their reciprocals
    sel_sb = persist.tile([66, H * D], BF16)       # head-broadcast selector
    bq_sb = persist.tile([P, PT], F32)
    bk_sb = persist.tile([P, PT], F32)
    bo_sb = persist.tile([P, E], F32)
    oA = persist.tile([P, QB // P, E], F32)        # out-proj partial (head pairs 0-2)

    wq_t = wpool.tile([P, EC, E], F8, tag="wq")
    wk_t = wpool.tile([P, EC, E], F8, tag="wk")
    wv_t = wpool.tile([P, EC, E], BF16, tag="wv")
    wo_t = wpool.tile([P, EC, E], BF16, tag="wo")
    xq_t = xpool.tile([P, EC, QB], F8, tag="xq")
    xk_t = xpool.tile([P, EC, S], F8, tag="xk")

    # --- input DMAs: one large transfer per tensor (per-chunk DMAs cost
    # ~625ns dispatch each and serialize the queue), spread over 3 queues ---
    nc.sync.dma_start(xq_t[:], xq[:])
    nc.sync.dma_start(wq_t[:], wq[:])
    nc.scalar.dma_start(xk_t[:], xk[:])
    nc.scalar.dma_start(wk_t[:], wk[:])
    nc.gpsimd.dma_start(bq_sb[:], bq[:])
    nc.gpsimd.dma_start(bk_sb[:], bk[:])
    nc.gpsimd.dma_start(wv_t[:], wv[:])
    nc.gpsimd.dma_start(bo_sb[:], bo[:])
    nc.gpsimd.dma_start(sel_sb[:], seld[:])

    nc.vector.memset(v_sb[:, :, :, D], 1.0)

    # --- PE warm-up: ~10 dummy matmuls so the pstate ramp (0.65->2.4GHz
    # after ~3us of continuous work) completes before the real projections ---
    wu = persist.tile([P, 512], BF16)
    nc.vector.memset(wu[:], 0.0)
    wups = psS.tile([P, 2, 512], F32, tag="sc", name="warm")
    for _ in range(24):
        nc.tensor.matmul(wups[:, 0, :], wu[:, 0:128], wu[:], start=True, stop=True)

    def emit_qproj(t):
        g, j = t // 2, t % 2
        ps = psS.tile([P, 2, 512], F32, tag="sc", name="qproj")
        for e in range(3):
            nc.tensor.matmul(ps[0:PR, 0, :], wq_t[:, 2 * e:2 * e + 2, t * PR:(t + 1) * PR],
                             xq_t[:, 2 * e:2 * e + 2, :],
                             start=(e == 0), stop=(e == 2), perf_mode=DR)
        nc.vector.tensor_scalar(qT8[0:PR, j, g, :], ps[0:PR, 0, :], QS / WS,
                                bq_sb[0:PR, t:t + 1], mult, add)

    def emit_kproj(t, n4list=range(NC4)):
        g, j = t // 2, t % 2
        for n4 in n4list:
            ps = psS.tile([P, 2, 512], F32, tag="sc", name="kproj")
            for e in range(3):
                nc.tensor.matmul(ps[0:PR, 0, :], wk_t[:, 2 * e:2 * e + 2, t * PR:(t + 1) * PR],
                                 xk_t[:, 2 * e:2 * e + 2, n4 * 512:(n4 + 1) * 512],
                                 start=(e == 0), stop=(e == 2), perf_mode=DR)
            nc.vector.tensor_scalar(kT8[0:PR, j, g, n4 * 512:(n4 + 1) * 512], ps[0:PR, 0, :],
                                    1.0 / WS, bk_sb[0:PR, t:t + 1], mult, add)

    def emit_vproj(kt):
        xv_t = xvpool.tile([P, EC, P], BF16, tag="xv")
        nc.gpsimd.dma_start(xv_t[:], xv[:, kt, :, :])
        psv = psS.tile([P, 2, 512], F32, tag="sc", name="vproj")
        fl = psv.rearrange("p a b -> p (a b)")
        for ec in range(EC):
            nc.tensor.matmul(fl[:, 0:512], xv_t[:, ec, :], wv_t[:, ec, 0:512],
                             start=(ec == 0), stop=(ec == EC - 1))
            nc.tensor.matmul(fl[:, 512:768], xv_t[:, ec, :], wv_t[:, ec, 512:768],
                             start=(ec == 0), stop=(ec == EC - 1))
        nc.vector.tensor_copy(v_sb[:, kt, :, 0:D], fl[:, 0:768].rearrange("p (h d) -> p h d", d=D))

    def _dnrow(h):
        return (h if h < 6 else 32 + (h - 6) if h < 10 else 64 + (h - 10))

    def flush_norm(r0, heads):
        n = len(heads)
        with nc.allow_low_precision(reason="1/denom in bf16: feeds a bf16 broadcast anyway"):
            nc.vector.reciprocal(drec2[r0:r0 + n, :], dens[r0:r0 + n, :])
        for h in heads:
            hp, i = h // 2, h % 2
            bc = psO.tile([D + 1, 512], F32, tag="po", name=f"bc{h}")
            nc.tensor.matmul(bc[0:D, :], sel_sb[r0:r0 + n, h * D:(h + 1) * D],
                             drec2[r0:r0 + n, :], start=True, stop=True)
            nc.vector.tensor_tensor(o_all[64 * i:64 * i + D, hp, :], bc[0:D, :],
                                    o_acc[0:D, h, :], mult)

    def emit_norm(hp):
        r = _dnrow(2 * hp)
        nc.gpsimd.dma_start(dens[r:r + 2, :],
                            o_acc[D:D + 1, 2 * hp:2 * hp + 2, :])

    def emit_scores(hp, kt):
        st = psS.tile([P, 2, 512], F32, tag="sc", name="sc")
        for i in range(2):
            h = 2 * hp + i
            g, m = h // 3, h % 3
            nc.tensor.matmul(st[:, i, :],
                             kT8[32 * m:32 * m + 32, :, g, kt * P:(kt + 1) * P],
                             qT8[32 * m:32 * m + 32, :, g, :],
                             start=True, stop=True, perf_mode=DR)
        ex = expool.tile([P, 2, 512], BF16, tag="ex")
        nc.scalar.activation(ex[:, :, :], st[:, :, :],
                             mybir.ActivationFunctionType.Exp, scale=1.0 / QS)
        return ex

    # --- attention scheduling ---
    # Blocks of key tiles; within each head pair, PV is software-pipelined
    # one key tile behind scores/exp so the in-order PE stream barely waits
    # on the Scalar engine.  Remaining projection work (v tiles, late q/k
    # tiles) sits in a FIFO queue pumped into PE slack; the idempotent
    # need_* helpers double as deadline enforcement at the use sites.
    from collections import deque

    BLOCKS = [(0, 4), (4, 10), (10, 16)]
    LASTB = len(BLOCKS) - 1
    done = set()

    def need_q(t):
        if ("q", t) not in done:
            done.add(("q", t))
            emit_qproj(t)

    def need_k(t):
        if ("k", t) not in done:
            done.add(("k", t))
            emit_kproj(t)

    def need_v(kt):
        if ("v", kt) not in done:
            done.add(("v", kt))
            emit_vproj(kt)

    def emit_pv(hp, kt, ex, o_ps, start, stop):
        need_v(kt)
        for i in range(2):
            nc.tensor.matmul(o_ps[i][:, :], v_sb[:, kt, 2 * hp + i, :], ex[:, i, :],
                             start=start, stop=stop)

    def emit_flush(b, hp, o_ps):
        for i in range(2):
            h = 2 * hp + i
            if b == 0:
                nc.vector.tensor_copy(o_acc[:, h, :], o_ps[i][:, :])
            else:
                nc.vector.tensor_tensor(o_acc[:, h, :], o_ps[i][:, :], o_acc[:, h, :], add)
        if b == LASTB:
            emit_norm(hp)

    ST = QB // P

    def outA_chunk(st4, hps, first):
        # partial output projection for one query chunk, accumulated into oA
        op = psS.tile([P, 2, 512], F32, tag="sc", name="oprojA")
        opf = op.rearrange("p a b -> p (a b)")
        for hp in hps:
            nc.tensor.matmul(opf[:, 0:512], o_all[:, hp, st4 * P:(st4 + 1) * P],
                             wo_t[:, hp, 0:512], start=(hp == hps[0]), stop=(hp == hps[-1]))
            nc.tensor.matmul(opf[:, 512:768], o_all[:, hp, st4 * P:(st4 + 1) * P],
                             wo_t[:, hp, 512:768], start=(hp == hps[0]), stop=(hp == hps[-1]))
        nc.vector.tensor_tensor(oA[:, st4, :], opf[:, 0:768],
                                bo_sb[:] if first else oA[:, st4, :], add)

    def emit_outB():
        # only head pair 5 remains after the final flush
        for st4 in range(ST):
            op = psS.tile([P, 2, 512], F32, tag="sc", name="oprojB")
            opf = op.rearrange("p a b -> p (a b)")
            hp = H // 2 - 1
            nc.tensor.matmul(opf[:, 0:512], o_all[:, hp, st4 * P:(st4 + 1) * P],
                             wo_t[:, hp, 0:512], start=True, stop=True)
            nc.tensor.matmul(opf[:, 512:768], o_all[:, hp, st4 * P:(st4 + 1) * P],
                             wo_t[:, hp, 512:768], start=True, stop=True)
            out_sb = outpool.tile([P, E], F32, tag="outsb")
            nc.vector.tensor_tensor(out_sb[:], opf[:, 0:768], oA[:, st4, :], add)
            nc.sync.dma_start(out[st4 * P:(st4 + 1) * P, :], out_sb[:])

    # startup projections: only what head pair 0 needs (heads 0,1 -> g0)
    need_q(0)
    need_q(1)
    need_k(0)
    need_k(1)

    pending = deque()
    for t in (2, 3):
        pending.append(lambda t=t: need_q(t))
        pending.append(lambda t=t: need_k(t))
    for kt in range(*BLOCKS[0]):
        pending.append(lambda kt=kt: need_v(kt))
    for t in range(4, PT):
        pending.append(lambda t=t: need_q(t))
        pending.append(lambda t=t: need_k(t))

    def pump():
        if pending:
            pending.popleft()()

    for b, (k0, k1) in enumerate(BLOCKS):
        if b < LASTB:
            for kt in range(*BLOCKS[b + 1]):
                pending.append(lambda kt=kt: need_v(kt))
        else:
            pending.append(lambda: nc.gpsimd.dma_start(wo_t[:], wo[:]))
        for hp in range(H // 2):
            for i in range(2):  # q/k deadline for this pair's heads
                g = (2 * hp + i) // 3
                need_q(2 * g)
                need_q(2 * g + 1)
                need_k(2 * g)
                need_k(2 * g + 1)
            o_ps = [psO.tile([D + 1, 512], F32, tag="po", name=f"o{b}_{hp}_{i}")
                    for i in range(2)]
            prev = None
            for kt in range(k0, k1):
                ex = emit_scores(hp, kt)
                if prev is not None:
                    emit_pv(hp, prev[0], prev[1], o_ps,
                            start=(prev[0] == k0), stop=False)
                prev = (kt, ex)
                pump()
            emit_pv(hp, prev[0], prev[1], o_ps, start=(prev[0] == k0), stop=True)
            emit_flush(b, hp, o_ps)
            pump()
            if b == LASTB:
                if hp == 3:
                    flush_norm(0, [0, 1, 2, 3, 4, 5])  # chain overlaps hp4's scores
                    for st4 in range(ST):
                        pending.append(lambda s=st4: outA_chunk(s, (0, 1, 2), True))
                elif hp == 4:
                    flush_norm(32, [6, 7, 8, 9])       # overlaps hp5's scores
                    for st4 in range(ST):
                        pending.append(lambda s=st4: outA_chunk(s, (3, 4), False))
                elif hp == H // 2 - 1:
                    flush_norm(64, [10, 11])
    while pending:
        pending.popleft()()
    emit_outB()


_NC_CACHE = None


def _get_nc():
    global _NC_CACHE
    if _NC_CACHE is None:
        _NC_CACHE = build_nc()
    return _NC_CACHE


def _dsplit_perm():
    """col i = t*96 + m*32 + dm  <-  head (3*(t//2)+m), d (32*(t%2)+dm)."""
    perm = np.empty(E, dtype=np.int64)
    i = 0
    for t in range(PT):
        g, j = t // 2, t % 2
        for m in range(3):
            for dm in range(32):
                perm[i] = (3 * g + m) * D + 32 * j + dm
                i += 1
    return perm


def make_in_maps(query, key_, value, Wq, bq, Wk, bk, Wv, bv, Wo, bo):
    """Host-side sharding + layout prep. Returns list of 8 input dicts."""
    import ml_dtypes
    BF = ml_dtypes.bfloat16
    F8NP = mybir.dt.np(F8)

    query = np.asarray(query, dtype=np.float32)
    key_ = np.asarray(key_, dtype=np.float32)
    value = np.asarray(value, dtype=np.float32)
    scale = 1.0 / np.sqrt(np.float32(D))
    perm = _dsplit_perm()

    wq_eff = np.ascontiguousarray(np.transpose(np.asarray(Wq, np.float32), (1, 0, 2)).reshape(E, E)) * scale
    wk_eff = np.ascontiguousarray(np.transpose(np.asarray(Wk, np.float32), (1, 0, 2)).reshape(E, E))
    def pack(a):  # [E, X] -> [P, EC, X] with partition-contiguous runs
        return np.ascontiguousarray(a.reshape(EC, P, -1).transpose(1, 0, 2))

    wq_f = pack((wq_eff[:, perm] * WS)).astype(F8NP)
    wk_f = pack((wk_eff[:, perm] * WS)).astype(F8NP)
    wv_f = pack(np.transpose(np.asarray(Wv, np.float32), (1, 0, 2)).reshape(E, E)).astype(BF)
    wo_f = pack(np.asarray(Wo, np.float32)).astype(BF)

    bq_eff = np.asarray(bq, np.float32).reshape(E) * scale * QS
    bk_eff = np.asarray(bk, np.float32).reshape(E)
    bq_f = np.zeros((P, PT), np.float32)
    bk_f = np.zeros((P, PT), np.float32)
    bq_f[0:PR, :] = bq_eff[perm].reshape(PT, PR).T
    bk_f[0:PR, :] = bk_eff[perm].reshape(PT, PR).T
    bv_f = np.asarray(bv, np.float32).reshape(E)
    bo_eff = np.tile((bv_f @ np.asarray(Wo, np.float32) + np.asarray(bo, np.float32)).reshape(1, E), (P, 1)).copy()

    sel_np = np.zeros((66, H * D), np.float32)
    for h in range(H):
        r = h if h < 6 else 32 + (h - 6) if h < 10 else 64 + (h - 10)
        sel_np[r, h * D:(h + 1) * D] = 1.0
    sel_np = sel_np.astype(BF)

    xk_t = [pack(key_[b].T).astype(F8NP) for b in range(B)]
    xv_t = [np.ascontiguousarray(value[b].T.reshape(EC, P, KT, P).transpose(1, 2, 0, 3)).astype(BF)
            for b in range(B)]

    in_maps = []
    for core in range(NCORES):
        b = core // (NCORES // B)
        qc = core % (NCORES // B)
        xq_f = pack(query[b, qc * QB:(qc + 1) * QB, :].T).astype(F8NP)
        in_maps.append({
            "xq": xq_f, "xk": xk_t[b], "xv": xv_t[b],
            "wq": wq_f, "wk": wk_f, "wv": wv_f, "wo": wo_f,
            "bq": bq_f, "bk": bk_f, "bo": bo_eff, "seld": sel_np,
        })
    return in_maps


def assemble(results):
    outp = np.empty((B, S, E), dtype=np.float32)
    for core in range(NCORES):
        b = core // (NCORES // B)
        qc = core % (NCORES // B)
        outp[b, qc * QB:(qc + 1) * QB, :] = results[core]["out"]
    return outp


def kernel(query, key_, value, Wq, bq, Wk, bk, Wv, bv, Wo, bo):
    nc = _get_nc()
    in_maps = make_in_maps(query, key_, value, Wq, bq, Wk, bk, Wv, bv, Wo, bo)
    res = run_bass_kernel_spmd(nc, in_maps, core_ids=list(range(NCORES)))
    return assemble(res.results)


# revision 29
# speedup vs baseline: 1.0727x; 1.0727x over previous
# Multi-head attention kernel for Trainium2, sharded over 8 NeuronCores.
#
# Sharding: core = (batch b, query-chunk qc). Each core handles QB=512 queries
# of one batch, all 12 heads, recomputing the K/V projections for its batch.
# (Cross-core collectives measured far too slow on this fabric: an AllGather
# of the projected K/V costs ~30us + ~50us launch overhead, much more than
# the redundant projection compute it would save.)
#
# Numerics (rel err budget 2e-2; measures ~9.5e-3 on HW):
#   - q/k path in fp8e4 with DoubleRow matmuls: inputs xq/xk and weights
#     wq/wk are fp8; weights scaled x64 to dodge the e4m3 subnormal range,
#     q stored x8; the 8x score scale is removed for free via the exp
#     activation's scale=1/8.
#   - v path, PV, and output projection stay bf16: fp8 noise on v/ex/o hits
#     the output undamped (sims at 1.5e-2+), so fp8 there is not safe.
#   - d-split layout: the host permutes W columns so projection PSUM rows
#     land directly as [32-partition blocks x 2 planes] (head = 3*g + m,
#     bases 0/32/64 - base 96 is not encodable for matmul operands); score
#     matmuls then contract K=64 as DoubleRow [32p x 2 planes], and the two
#     heads of a pair run CONCURRENTLY on disjoint PE row groups.
#
# Schedule (exp is ~107us of Scalar work; PE stream is ~150us and is the
# binding engine, so the goal is to keep PE saturated and never let its
# in-order stream wait on the Scalar engine):
#   - all inputs are host-packed into SBUF layouts so each is ONE DMA of
#     long per-partition runs (sub-KB-run DMAs measured ~47GB/s); ~24 dummy
#     warm-up matmuls bridge the input-DMA latency so the PE clock ramp
#     (0.65->2.4GHz, reset by any idle moment) is complete when real work
#     starts.
#   - kt-blocked attention (blocks of 4/6/6 key tiles x 6 head pairs):
#     o (+ ones-column denominator row) accumulates in PSUM per block and is
#     flushed into an SBUF f32 accumulator on DVE.  PSUM: 3 score slots
#     [128,2,512] + 2 o slots [65,512] = 8 banks.
#   - within a head pair, PV is software-pipelined one key tile behind
#     scores/exp; remaining projection work (v tiles, late q/k tiles) sits
#     in a FIFO queue pumped between score groups, with idempotent need_*
#     helpers as deadline enforcement.  FIFO order guarantees a block's v
#     tiles are emitted before that block's PVs.
#   - softmax normalization (denominator gather + one batched reciprocal
#     per 6 heads + PE selector-broadcast + DVE multiply) and the first
#     half of the output projection are emitted inside the last block's
#     Scalar-bound stretch, so the tail after the final exp is only the
#     second half of the output projection.
#
import numpy as np
from contextlib import ExitStack

import concourse.bass as bass
import concourse.mybir as mybir
import concourse.tile as tile
from concourse import bacc
from concourse.bass_utils import run_bass_kernel_spmd

F32 = mybir.dt.float32
BF16 = mybir.dt.bfloat16
F8 = mybir.dt.float8e4
DR = mybir.MatmulPerfMode.DoubleRow
P = 128
E = 768
S = 2048
B = 2
H = 12
D = 64
QB = 512          # queries per core
NCORES = 8
EC = E // P       # 6 e-chunks
KT = S // P       # 16 key tiles
MT = E // P       # 6 e-chunks of proj output
NG = 4            # d-split head groups (head = 3*g + m, m in 0..2)
PT = 8            # projection tiles of 96 rows (bases 96+ are not encodable)
PR = 96           # rows per projection tile
NC4 = S // 512    # 4 n-slices of k
KB = 4            # key tiles per attention block
NBLK = KT // KB   # 4 blocks
WS = 64.0         # fp8 weight scale (avoids e4m3 subnormals)
QS = 8.0          # qT8 storage scale; removed via exp scale


def build_nc():
    nc = bacc.Bacc("TRN2", debug=False)

    # all inputs pre-packed host-side into SBUF layouts so every DMA moves
    # long contiguous runs per partition (sub-KB runs measure ~47GB/s)
    xq = nc.dram_tensor("xq", (P, EC, QB), F8, kind="ExternalInput")
    xk = nc.dram_tensor("xk", (P, EC, S), F8, kind="ExternalInput")
    xv = nc.dram_tensor("xv", (P, KT, EC, P), BF16, kind="ExternalInput")
    wq = nc.dram_tensor("wq", (P, EC, E), F8, kind="ExternalInput")   # d-split cols, x(1/sqrt(D))x64
    wk = nc.dram_tensor("wk", (P, EC, E), F8, kind="ExternalInput")   # d-split cols, x64
    wv = nc.dram_tensor("wv", (P, EC, E), BF16, kind="ExternalInput")
    wo = nc.dram_tensor("wo", (P, EC, E), BF16, kind="ExternalInput")
    bq = nc.dram_tensor("bq", (P, PT), F32, kind="ExternalInput")   # d-split rows, x8
    bk = nc.dram_tensor("bk", (P, PT), F32, kind="ExternalInput")   # d-split rows
    bo = nc.dram_tensor("bo", (P, E), F32, kind="ExternalInput")    # bv@Wo + bo, broadcast
    seld = nc.dram_tensor("seld", (66, H * D), BF16, kind="ExternalInput")  # head-broadcast selector
    out = nc.dram_tensor("out", (QB, E), F32, kind="ExternalOutput")

    with tile.TileContext(nc) as tc:
        with ExitStack() as ctx:
            _emit(ctx, tc, nc, xq, xk, xv, wq, wk, wv, wo, bq, bk, bo, seld, out)
    nc.compile()
    return nc


def _emit(ctx, tc, nc, xq, xk, xv, wq, wk, wv, wo, bq, bk, bo, seld, out):
    mult = mybir.AluOpType.mult
    add = mybir.AluOpType.add

    persist = ctx.enter_context(tc.tile_pool(name="persist", bufs=1))
    wpool = ctx.enter_context(tc.tile_pool(name="wpool", bufs=1))
    xpool = ctx.enter_context(tc.tile_pool(name="xpool", bufs=1))
    xvpool = ctx.enter_context(tc.tile_pool(name="xvpool", bufs=3))
    expool = ctx.enter_context(tc.tile_pool(name="expool", bufs=16))
    outpool = ctx.enter_context(tc.tile_pool(name="outpool", bufs=2))
    psS = ctx.enter_context(tc.tile_pool(name="psS", bufs=3, space="PSUM"))  # [128,2,512] scores/proj
    psO = ctx.enter_context(tc.tile_pool(name="psO", bufs=2, space="PSUM"))  # [65,512] o accum / bc

    # persistent SBUF
    qT8 = persist.tile([P, 2, NG, QB], F8)         # [32m+dm, plane j, group g, q] = 8*q
    kT8 = persist.tile([P, 2, NG, S], F8)          # [32m+dm, j, g, key] = k
    v_sb = persist.tile([P, KT, H, D + 1], BF16)   # v + ones column per head
    o_acc = persist.tile([D + 1, H, QB], F32)      # flushed o (+denominator in row D)
    o_all = persist.tile([P, H // 2, QB], BF16)    # normalized o, pairs in partition halves
    dens = persist.tile([66, QB], F32)             # denominators: rows 0-5 / 32-35 / 64-65
    drec2 = persist.tile([66, QB], BF16)           # their reciprocals
    sel_sb = persist.tile([66, H * D], BF16)       # head-broadcast selector
    bq_sb = persist.tile([P, PT], F32)
    bk_sb = persist.tile([P, PT], F32)
    bo_sb = persist.tile([P, E], F32)
    oA = persist.tile([P, QB // P, E], F32)        # out-proj partial (head pairs 0-2)

    wq_t = wpool.tile([P, EC, E], F8, tag="wq")
    wk_t = wpool.tile([P, EC, E], F8, tag="wk")
    wv_t = wpool.tile([P, EC, E], BF16, tag="wv")
    wo_t = wpool.tile([P, EC, E], BF16, tag="wo")
    xq_t = xpool.tile([P, EC, QB], F8, tag="xq")
    xk_t = xpool.tile([P, EC, S], F8, tag="xk")

    # --- input DMAs: one large transfer per tensor (per-chunk DMAs cost
    # ~625ns dispatch each and serialize the queue), spread over 3 queues ---
    nc.sync.dma_start(xq_t[:], xq[:])
    nc.sync.dma_start(wq_t[:], wq[:])
    nc.scalar.dma_start(xk_t[:], xk[:])
    nc.scalar.dma_start(wk_t[:], wk[:])
    nc.gpsimd.dma_start(bq_sb[:], bq[:])
    nc.gpsimd.dma_start(bk_sb[:], bk[:])
    nc.gpsimd.dma_start(wv_t[:], wv[:])
    nc.gpsimd.dma_start(bo_sb[:], bo[:])
    nc.gpsimd.dma_start(sel_sb[:], seld[:])

    nc.vector.memset(v_sb[:, :, :, D], 1.0)

    # --- PE warm-up: ~10 dummy matmuls so the pstate ramp (0.65->2.4GHz
    # after ~3us of continuous work) completes before the real projections ---
    wu = persist.tile([P, 512], BF16)
    nc.vector.memset(wu[:], 0.0)
    wups = psS.tile([P, 2, 512], F32, tag="sc", name="warm")
    for _ in range(24):
        nc.tensor.matmul(wups[:, 0, :], wu[:, 0:128], wu[:], start=True, stop=True)

    def emit_qproj(t):
        g, j = t // 2, t % 2
        ps = psS.tile([P, 2, 512], F32, tag="sc", name="qproj")
        for e in range(3):
            nc.tensor.matmul(ps[0:PR, 0, :], wq_t[:, 2 * e:2 * e + 2, t * PR:(t + 1) * PR],
                             xq_t[:, 2 * e:2 * e + 2, :],
                             start=(e == 0), stop=(e == 2), perf_mode=DR)
        nc.vector.tensor_scalar(qT8[0:PR, j, g, :], ps[0:PR, 0, :], QS / WS,
                                bq_sb[0:PR, t:t + 1], mult, add)

    def emit_kproj(t, n4list=range(NC4)):
        g, j = t // 2, t % 2
        for n4 in n4list:
            ps = psS.tile([P, 2, 512], F32, tag="sc", name="kproj")
            for e in range(3):
                nc.tensor.matmul(ps[0:PR, 0, :], wk_t[:, 2 * e:2 * e + 2, t * PR:(t + 1) * PR],
                                 xk_t[:, 2 * e:2 * e + 2, n4 * 512:(n4 + 1) * 512],
                                 start=(e == 0), stop=(e == 2), perf_mode=DR)
            nc.vector.tensor_scalar(kT8[0:PR, j, g, n4 * 512:(n4 + 1) * 512], ps[0:PR, 0, :],
                                    1.0 / WS, bk_sb[0:PR, t:t + 1], mult, add)

    def emit_vproj(kt):
        xv_t = xvpool.tile([P, EC, P], BF16, tag="xv")
        nc.gpsimd.dma_start(xv_t[:], xv[:, kt, :, :])
        psv = psS.tile([P, 2, 512], F32, tag="sc", name="vproj")
        fl = psv.rearrange("p a b -> p (a b)")
        for ec in range(EC):
            nc.tensor.matmul(fl[:, 0:512], xv_t[:, ec, :], wv_t[:, ec, 0:512],
                             start=(ec == 0), stop=(ec == EC - 1))
            nc.tensor.matmul(fl[:, 512:768], xv_t[:, ec, :], wv_t[:, ec, 512:768],
                             start=(ec == 0), stop=(ec == EC - 1))
        nc.vector.tensor_copy(v_sb[:, kt, :, 0:D], fl[:, 0:768].rearrange("p (h d) -> p h d", d=D))

    def _dnrow(h):
        return (h if h < 6 else 32 + (h - 6) if h < 10 else 64 + (h - 10))

    def flush_norm(r0, heads):
        n = len(heads)
        with nc.allow_low_precision(reason="1/denom in bf16: feeds a bf16 broadcast anyway"):
            nc.vector.reciprocal(drec2[r0:r0 + n, :], dens[r0:r0 + n, :])
        for h in heads:
            hp, i = h // 2, h % 2
            bc = psO.tile([D + 1, 512], F32, tag="po", name=f"bc{h}")
            nc.tensor.matmul(bc[0:D, :], sel_sb[r0:r0 + n, h * D:(h + 1) * D],
                             drec2[r0:r0 + n, :], start=True, stop=True)
            nc.vector.tensor_tensor(o_all[64 * i:64 * i + D, hp, :], bc[0:D, :],
                                    o_acc[0:D, h, :], mult)

    def emit_norm(hp):
        r = _dnrow(2 * hp)
        nc.gpsimd.dma_start(dens[r:r + 2, :],
                            o_acc[D:D + 1, 2 * hp:2 * hp + 2, :])

    def emit_scores(hp, kt):
        st = psS.tile([P, 2, 512], F32, tag="sc", name="sc")
        for i in range(2):
            h = 2 * hp + i
            g, m = h // 3, h % 3
            nc.tensor.matmul(st[:, i, :],
                             kT8[32 * m:32 * m + 32, :, g, kt * P:(kt + 1) * P],
                             qT8[32 * m:32 * m + 32, :, g, :],
                             start=True, stop=True, perf_mode=DR)
        ex = expool.tile([P, 2, 512], BF16, tag="ex")
        nc.scalar.activation(ex[:, :, :], st[:, :, :],
                             mybir.ActivationFunctionType.Exp, scale=1.0 / QS)
        return ex

    # --- attention scheduling ---
    # Blocks of key tiles; within each head pair, PV is software-pipelined
    # one key tile behind scores/exp so the in-order PE stream barely waits
    # on the Scalar engine.  Remaining projection work (v tiles, late q/k
    # tiles) sits in a FIFO queue pumped into PE slack; the idempotent
    # need_* helpers double as deadline enforcement at the use sites.
    from collections import deque

    BLOCKS = [(0, 4), (4, 10), (10, 16)]
    LASTB = len(BLOCKS) - 1
    done = set()

    def need_q(t):
        if ("q", t) not in done:
            done.add(("q", t))
            emit_qproj(t)

    def need_k(t):
        if ("k", t) not in done:
            done.add(("k", t))
            emit_kproj(t)

    def need_v(kt):
        if ("v", kt) not in done:
            done.add(("v", kt))
            emit_vproj(kt)

    def emit_pv(hp, kt, ex, o_ps, start, stop):
        need_v(kt)
        for i in range(2):
            nc.tensor.matmul(o_ps[i][:, :], v_sb[:, kt, 2 * hp + i, :], ex[:, i, :],
                             start=start, stop=stop)

    def emit_flush(b, hp, o_ps):
        for i in range(2):
            h = 2 * hp + i
            if b == 0:
                nc.vector.tensor_copy(o_acc[:, h, :], o_ps[i][:, :])
            else:
                nc.vector.tensor_tensor(o_acc[:, h, :], o_ps[i][:, :], o_acc[:, h, :], add)
        if b == LASTB:
            emit_norm(hp)

    ST = QB // P

    def emit_outA():
        # first half of the output projection (head pairs 0-2 + bias): runs
        # inside the last block's Scalar-bound stretch as real PE filler
        for st4 in range(ST):
            op = psS.tile([P, 2, 512], F32, tag="sc", name="oprojA")
            opf = op.rearrange("p a b -> p (a b)")
            for hp in range(3):
                nc.tensor.matmul(opf[:, 0:512], o_all[:, hp, st4 * P:(st4 + 1) * P],
                                 wo_t[:, hp, 0:512], start=(hp == 0), stop=(hp == 2))
                nc.tensor.matmul(opf[:, 512:768], o_all[:, hp, st4 * P:(st4 + 1) * P],
                                 wo_t[:, hp, 512:768], start=(hp == 0), stop=(hp == 2))
            nc.vector.tensor_tensor(oA[:, st4, :], opf[:, 0:768], bo_sb[:], add)

    def emit_outB():
        for st4 in range(ST):
            op = psS.tile([P, 2, 512], F32, tag="sc", name="oprojB")
            opf = op.rearrange("p a b -> p (a b)")
            for hp in range(3, H // 2):
                nc.tensor.matmul(opf[:, 0:512], o_all[:, hp, st4 * P:(st4 + 1) * P],
                                 wo_t[:, hp, 0:512], start=(hp == 3), stop=(hp == H // 2 - 1))
                nc.tensor.matmul(opf[:, 512:768], o_all[:, hp, st4 * P:(st4 + 1) * P],
                                 wo_t[:, hp, 512:768], start=(hp == 3), stop=(hp == H // 2 - 1))
            out_sb = outpool.tile([P, E], F32, tag="outsb")
            nc.vector.tensor_tensor(out_sb[:], opf[:, 0:768], oA[:, st4, :], add)
            nc.sync.dma_start(out[st4 * P:(st4 + 1) * P, :], out_sb[:])

    # startup projections: only what head pair 0 needs (heads 0,1 -> g0)
    need_q(0)
    need_q(1)
    need_k(0)
    need_k(1)

    pending = deque()
    for t in (2, 3):
        pending.append(lambda t=t: need_q(t))
        pending.append(lambda t=t: need_k(t))
    for kt in range(*BLOCKS[0]):
        pending.append(lambda kt=kt: need_v(kt))
    for t in range(4, PT):
        pending.append(lambda t=t: need_q(t))
        pending.append(lambda t=t: need_k(t))

    def pump():
        if pending:
            pending.popleft()()

    for b, (k0, k1) in enumerate(BLOCKS):
        if b < LASTB:
            for kt in range(*BLOCKS[b + 1]):
                pending.append(lambda kt=kt: need_v(kt))
        else:
            pending.append(lambda: nc.gpsimd.dma_start(wo_t[:], wo[:]))
        for hp in range(H // 2):
            for i in range(2):  # q/k deadline for this pair's heads
                g = (2 * hp + i) // 3
                need_q(2 * g)
                need_q(2 * g + 1)
                need_k(2 * g)
                need_k(2 * g + 1)
            o_ps = [psO.tile([D + 1, 512], F32, tag="po", name=f"o{b}_{hp}_{i}")
                    for i in range(2)]
            prev = None
            for kt in range(k0, k1):
                ex = emit_scores(hp, kt)
                if prev is not None:
                    emit_pv(hp, prev[0], prev[1], o_ps,
                            start=(prev[0] == k0), stop=False)
                prev = (kt, ex)
                pump()
            emit_pv(hp, prev[0], prev[1], o_ps, start=(prev[0] == k0), stop=True)
            emit_flush(b, hp, o_ps)
            pump()
            if b == LASTB:
                if hp == 3:
                    flush_norm(0, [0, 1, 2, 3, 4, 5])  # chain overlaps hp4's scores
                elif hp == 4:
                    emit_outA()
                elif hp == H // 2 - 1:
                    flush_norm(32, [6, 7, 8, 9])
                    flush_norm(64, [10, 11])
    while pending:
        pending.popleft()()
    emit_outB()


_NC_CACHE = None


def _get_nc():
    global _NC_CACHE
    if _NC_CACHE is None:
        _NC_CACHE = build_nc()
    return _NC_CACHE


def _dsplit_perm():
    """col i = t*96 + m*32 + dm  <-  head (3*(t//2)+m), d (32*(t%2)+dm)."""
    perm = np.empty(E, dtype=np.int64)
    i = 0
    for t in range(PT):
        g, j = t // 2, t % 2
        for m in range(3):
            for dm in range(32):
                perm[i] = (3 * g + m) * D + 32 * j + dm
                i += 1
    return perm


def make_in_maps(query, key_, value, Wq, bq, Wk, bk, Wv, bv, Wo, bo):
    """Host-side sharding + layout prep. Returns list of 8 input dicts."""
    import ml_dtypes
    BF = ml_dtypes.bfloat16
    F8NP = mybir.dt.np(F8)

    query = np.asarray(query, dtype=np.float32)
    key_ = np.asarray(key_, dtype=np.float32)
    value = np.asarray(value, dtype=np.float32)
    scale = 1.0 / np.sqrt(np.float32(D))
    perm = _dsplit_perm()

    wq_eff = np.ascontiguousarray(np.transpose(np.asarray(Wq, np.float32), (1, 0, 2)).reshape(E, E)) * scale
    wk_eff = np.ascontiguousarray(np.transpose(np.asarray(Wk, np.float32), (1, 0, 2)).reshape(E, E))
    def pack(a):  # [E, X] -> [P, EC, X] with partition-contiguous runs
        return np.ascontiguousarray(a.reshape(EC, P, -1).transpose(1, 0, 2))

    wq_f = pack((wq_eff[:, perm] * WS)).astype(F8NP)
    wk_f = pack((wk_eff[:, perm] * WS)).astype(F8NP)
    wv_f = pack(np.transpose(np.asarray(Wv, np.float32), (1, 0, 2)).reshape(E, E)).astype(BF)
    wo_f = pack(np.asarray(Wo, np.float32)).astype(BF)

    bq_eff = np.asarray(bq, np.float32).reshape(E) * scale * QS
    bk_eff = np.asarray(bk, np.float32).reshape(E)
    bq_f = np.zeros((P, PT), np.float32)
    bk_f = np.zeros((P, PT), np.float32)
    bq_f[0:PR, :] = bq_eff[perm].reshape(PT, PR).T
    bk_f[0:PR, :] = bk_eff[perm].reshape(PT, PR).T
    bv_f = np.asarray(bv, np.float32).reshape(E)
    bo_eff = np.tile((bv_f @ np.asarray(Wo, np.float32) + np.asarray(bo, np.float32)).reshape(1, E), (P, 1)).copy()

    sel_np = np.zeros((66, H * D), np.float32)
    for h in range(H):
        r = h if h < 6 else 32 + (h - 6) if h < 10 else 64 + (h - 10)
        sel_np[r, h * D:(h + 1) * D] = 1.0
    sel_np = sel_np.astype(BF)

    xk_t = [pack(key_[b].T).astype(F8NP) for b in range(B)]
    xv_t = [np.ascontiguousarray(value[b].T.reshape(EC, P, KT, P).transpose(1, 2, 0, 3)).astype(BF)
            for b in range(B)]

    in_maps = []
    for core in range(NCORES):
        b = core // (NCORES // B)
        qc = core % (NCORES // B)
        xq_f = pack(query[b, qc * QB:(qc + 1) * QB, :].T).astype(F8NP)
        in_maps.append({
            "xq": xq_f, "xk": xk_t[b], "xv": xv_t[b],
            "wq": wq_f, "wk": wk_f, "wv": wv_f, "wo": wo_f,
            "bq": bq_f, "bk": bk_f, "bo": bo_eff, "seld": sel_np,
        })
    return in_maps


def assemble(results):
    outp = np.empty((B, S, E), dtype=np.float32)
    for core in range(NCORES):
        b = core // (NCORES // B)
        qc = core % (NCORES // B)
        outp[b, qc * QB:(qc + 1) * QB, :] = results[core]["out"]
    return outp


def kernel(query, key_, value, Wq, bq, Wk, bk, Wv, bv, Wo, bo):
    nc = _get_nc()
    in_maps = make_in_maps(query, key_, value, Wq, bq, Wk, bk, Wv, bv, Wo, bo)
    res = run_bass_kernel_spmd(nc, in_maps, core_ids=list(range(NCORES)))
    return assemble(res.results)
